# revision 35
# baseline (speedup 1.0000x reference)
"""Trainium2 Bass kernel for nn_BinaryLinear (binarized linear layer).

Computes: out = sign(x) @ sign(W).T + bias
  x: [8192, 4096] f32, W: [4096, 4096] f32, bias: [4096] f32 -> out [8192, 4096] f32
  sign(v) = +1 if v >= 0 else -1

Sharding: 4x2 grid over 8 NeuronCores - batch split 4 ways (2048 rows each),
W rows (out_features) split 2 ways (2048 each). Each core computes a disjoint
[2048, 2048] output block; no collectives.

Host-side staging (inside kernel(), part of sharding): each operand shard is
shipped K-major (transposed) as the f32 TOP BYTE (sign + 7 exponent bits;
-0.0 normalized to +0.0 first), pre-tiled so each DMA unit is one contiguous
256 KiB block ([128 partitions, 4 k-tiles, 512 rows]). byte < 128 <=> v >= 0,
so the device-side binarize sees exactly the signs the reference sees. This
cuts HBM input traffic 4x (32 MiB/core total vs 80) and removes the on-device
transpose entirely - the PE runs a pure DoubleRow fp8 matmul stream.

Device-side (per core), exact (rel err 0 vs the reference):
  1. Prep: DMA one 256 KiB u8 unit into a staging ring, DVE binarizes to
     fp8 +-0.5 in one op ((byte<128) - 0.5) straight into the resident
     K-major per-super operand tensors. No PE transposes, no PSUM round-trip.
     The DVE runs ONLY binarizes, so across loop passes the next pass's
     operand prep is never queued behind an end-of-pass epilogue op.
  2. Matmul: fp8 DoubleRow K-accumulated PE matmuls computing the TRANSPOSED
     output (out_features on partitions): psum = exact_int/4 (quarter-ints
     bounded by 1024 accumulate exactly in fp32 PSUM).
  3. Epilogue: single fused ACT op per tile: out = psum*4 + bias, with bias
     as a per-partition operand (out_features is the partition dim). DMA out;
     the host transposes each [N, M] shard back during unshard.
Blocks are ordered to retire w-super 0 and x-super 0 as early as possible so
the next loop pass's first binarizes get a long runway.
"""

import os

import numpy as np

import concourse.bacc as bacc
import concourse.mybir as mybir
import concourse.tile as tile
from concourse.alu_op_type import AluOpType
from concourse.bass_utils import run_bass_kernel_spmd

P = 128
N_CORES = 8
M_SPLIT = 4  # batch split
N_SPLIT = 2  # out_features split

# Full-problem shapes (hardcoded per harness contract)
BATCH = 8192
IN_FEATURES = 4096
OUT_FEATURES = 4096

F32 = mybir.dt.float32
BF16 = mybir.dt.bfloat16
FP8 = mybir.dt.float8e4

SUPER = 512  # rows per operand super == matmul moving free dim == PSUM bank
KG = 4  # k-tiles per prep unit (unit = contiguous [P, KG, SUPER] bf16)


def build_nc(
    M,
    K,
    N,
    n_cores=N_CORES,
    double_row=True,
    repeat=1,
    timing_variant=False,
    body_parts="all",  # "all" | "mm" | "prep" | "prep_nodma" | "all_nodma"
    stage_bufs=12,
    mm_bufs=8,
    out_bufs=6,
    kg=KG,  # k-tiles per prep unit (host _pretile must match module KG)
    dma_split=1,  # parallel dma_starts per stage unit (cuts per-unit latency)
    u8=True,  # operands shipped as 1-byte f32 top-byte slices (sign+exp)
    block_order="retire0",  # "retire0" | "natural"
    stage_eng="sync",  # engine queue issuing stage-load DMAs
    act_binarize=False,  # binarize w-supers 1.. on ACT (Sign -> +-1) to offload DVE
):
    """Build the per-core kernel. DRAM inputs (pre-tiled on host):
      xT_shard [M/SUPER * KT/KG * P, KG*SUPER] bf16   (moving operand)
      wT_shard [N/SUPER * KT/KG * P, KG*SUPER] bf16   (stationary operand)
      bias_c   [P, N/P] f32  (bias_c[p, j] = bias[j*P + p])
    -> outT_shard [N, M] f32  (transposed output block)
    """
    assert double_row, "v3 kernel is DoubleRow-only"
    assert K % (P * kg) == 0 and M % SUPER == 0 and N % SUPER == 0
    KT = K // P  # k-tiles (32)
    KP = KT // 2  # k-pairs per psum accumulation (16)
    UG = KT // kg  # prep units per super (8)
    MS_ = M // SUPER  # x supers / moving panels (4)
    NS_ = N // SUPER  # w supers (4)
    RB = SUPER // P  # o-tiles per w super (4)
    NT = N // P  # bias columns (16)

    IN_DT = mybir.dt.uint8 if u8 else BF16

    nc = bacc.Bacc(
        "TRN2", target_bir_lowering=False, debug=False, num_devices=n_cores
    )
    if timing_variant:
        xT_in = nc.dram_tensor("xT_int", [MS_ * UG * P, kg * SUPER], IN_DT).ap()
        wT_in = nc.dram_tensor("wT_int", [NS_ * UG * P, kg * SUPER], IN_DT).ap()
        b_in = nc.dram_tensor("b_int", [P, NT], F32).ap()
        out = nc.dram_tensor("outT_int", [N, M], F32).ap()
        dummy_out = nc.dram_tensor("dummy_out", [P, 16], F32, kind="ExternalOutput").ap()
    else:
        xT_in = nc.dram_tensor(
            "xT_shard", [MS_ * UG * P, kg * SUPER], IN_DT, kind="ExternalInput"
        ).ap()
        wT_in = nc.dram_tensor(
            "wT_shard", [NS_ * UG * P, kg * SUPER], IN_DT, kind="ExternalInput"
        ).ap()
        b_in = nc.dram_tensor("bias_c", [P, NT], F32, kind="ExternalInput").ap()
        out = nc.dram_tensor("outT_shard", [N, M], F32, kind="ExternalOutput").ap()

    with tile.TileContext(nc) as tc:
        with (
            tc.tile_pool(name="const", bufs=1) as const,
            tc.tile_pool(name="resid", bufs=1) as resid,
            tc.tile_pool(name="stage", bufs=stage_bufs) as stage_pool,
            tc.tile_pool(name="mm", bufs=mm_bufs, space="PSUM") as mm_pool,
            tc.tile_pool(name="outp", bufs=out_bufs) as out_pool,
        ):
            bias_sb = const.tile([P, NT], F32, name="bias_sb", tag="bias_sb")
            nc.sync.dma_start(bias_sb, b_in)

            xT = [
                resid.tile([P, KT, SUPER], FP8, name=f"xT{s}", tag=f"xT{s}")
                for s in range(MS_)
            ]
            wT = [
                resid.tile([P, KT, SUPER], FP8, name=f"wT{s}", tag=f"wT{s}")
                for s in range(NS_)
            ]

            if body_parts == "mm":
                for t in xT + wT:
                    nc.any.memset(t, 0.5)

            c128 = None
            if act_binarize:
                c128 = const.tile([P, 1], F32, name="c128", tag="c128")
                nc.any.memset(c128, 128.0)

            fixed_stage = None
            if body_parts in ("prep_nodma", "all_nodma"):
                fixed_stage = const.tile(
                    [P, kg * SUPER], IN_DT, name="fixed_stage", tag="fixed_stage"
                )
                nc.any.memset(fixed_stage, 1 if u8 else 0.25)

            def prep_unit(kind, s, ug):
                """Load unit (super s, k-group ug) and binarize to fp8 +-0.5
                into xT[s][:, ug*kg:(ug+1)*kg, :] (resp. wT)."""
                src_ap = xT_in if kind == "x" else wT_in
                dst = (xT if kind == "x" else wT)[s]
                r0 = (s * UG + ug) * P
                if fixed_stage is not None:
                    st = fixed_stage
                else:
                    st = stage_pool.tile(
                        [P, kg * SUPER], IN_DT, name="stage", tag="stage"
                    )
                    cw = kg * SUPER // dma_split
                    eng = getattr(nc, stage_eng)
                    for d in range(dma_split):
                        eng.dma_start(
                            st[:, d * cw : (d + 1) * cw],
                            src_ap[r0 : r0 + P, d * cw : (d + 1) * cw],
                        )
                if u8:
                    if act_binarize and kind == "w" and s > 0:
                        # ACT path: Sign(128 - byte) = +-1 (w operands at +-1,
                        # x at +-0.5 -> psum = S/2, epilogue scale 2). Only
                        # w1.. go here: w0 must be ready at pass start, and
                        # ACT's queue tail is end-gated by the last epilogue.
                        nc.scalar.activation(
                            dst[:, ug * kg : (ug + 1) * kg, :],
                            st,
                            mybir.ActivationFunctionType.Sign,
                            scale=-1.0,
                            bias=c128[:, 0:1],
                        )
                    else:
                        # byte = f32 top byte; bit7 = sign: byte < 128 <=> v >= 0
                        nc.vector.tensor_scalar(
                            out=dst[:, ug * kg : (ug + 1) * kg, :],
                            in0=st,
                            scalar1=128,
                            scalar2=0.5,
                            op0=AluOpType.is_lt,
                            op1=AluOpType.subtract,
                        )
                else:
                    nc.vector.tensor_scalar(
                        out=dst[:, ug * kg : (ug + 1) * kg, :],
                        in0=st,
                        scalar1=0.0,
                        scalar2=0.5,
                        op0=AluOpType.is_ge,
                        op1=AluOpType.subtract,
                    )

            def mm_group(os_, ms, ot):
                """16 accumulating DR MMs for one [128(o), SUPER(b)] psum."""
                psum = mm_pool.tile([P, SUPER], F32, name="mmps", tag="mmps")
                for kp in range(KP):
                    nc.tensor.matmul(
                        psum,
                        lhsT=wT[os_][:, 2 * kp : 2 * kp + 2, ot * P : (ot + 1) * P],
                        rhs=xT[ms][:, 2 * kp : 2 * kp + 2, :],
                        start=(kp == 0),
                        stop=(kp == KP - 1),
                        perf_mode=mybir.MatmulPerfMode.DoubleRow,
                    )
                return psum

            def epi_group(os_, ms, ot, psum):
                ob = out_pool.tile([P, SUPER], F32, name="ob", tag="ob")
                # psum holds exact_int/4 (or /2 for ACT-binarized +-1 w
                # supers); one fused op: out = psum*scale + bias (bias is
                # per-partition in this orientation)
                scl = 2.0 if (act_binarize and os_ > 0) else 4.0
                nc.scalar.activation(
                    ob,
                    psum,
                    mybir.ActivationFunctionType.Identity,
                    scale=scl,
                    bias=bias_sb[:, os_ * RB + ot : os_ * RB + ot + 1],
                )
                r0 = os_ * SUPER + ot * P
                nc.sync.dma_start(
                    out[r0 : r0 + P, ms * SUPER : (ms + 1) * SUPER], ob
                )

            # prep order: w0/x0 k-interleaved, then x1.. (needed by the first
            # block sweep), then w1..
            first_q = [
                (kind, 0, ug) for ug in range(UG) for kind in ("w", "x")
            ]
            rest_q = [("x", s, ug) for s in range(1, MS_) for ug in range(UG)] + [
                ("w", s, ug) for s in range(1, NS_) for ug in range(UG)
            ]
            prep_q_all = first_q + rest_q

            def emit_body():
                if body_parts in ("prep", "prep_nodma"):
                    for unit in prep_q_all:
                        prep_unit(*unit)
                    return
                if body_parts == "mm":
                    for os_ in range(NS_):
                        for ms in range(MS_):
                            for ot in range(RB):
                                psum = mm_group(os_, ms, ot)
                                epi_group(os_, ms, ot, psum)
                    return

                q = list(prep_q_all)
                totals = {}
                for kind, s, ug in q:
                    totals[(kind, s)] = totals.get((kind, s), 0) + 1
                done = {}

                def emit_prep():
                    kind, s, ug = q.pop(0)
                    prep_unit(kind, s, ug)
                    done[(kind, s)] = done.get((kind, s), 0) + 1

                def deps_met(keys):
                    return all(done.get(k, 0) == totals[k] for k in keys)

                # Retire w-super 0 and x-super 0 as early as possible: the
                # next loop pass's first MM blocks need them re-binarized,
                # and that binarize can only start once the last reader in
                # THIS pass is done.
                blocks = []
                if block_order == "retire0":
                    for ms in range(MS_):
                        for ot in range(RB):
                            blocks.append((0, ms, ot))
                    for os_ in range(1, NS_):
                        for ot in range(RB):
                            blocks.append((os_, 0, ot))
                    for os_ in range(1, NS_):
                        for ms in range(1, MS_):
                            for ot in range(RB):
                                blocks.append((os_, ms, ot))
                else:
                    for os_ in range(NS_):
                        for ms in range(MS_):
                            for ot in range(RB):
                                blocks.append((os_, ms, ot))

                per_block = (len(q) + len(blocks) - 1) // len(blocks)
                pending = None
                for os_, ms, ot in blocks:
                    need = [("w", os_), ("x", ms)]
                    while q and not deps_met(need):
                        emit_prep()
                    psum = mm_group(os_, ms, ot)
                    want = per_block
                    while q and want > 0:
                        emit_prep()
                        want -= 1
                    if pending is not None:
                        epi_group(*pending)
                    pending = (os_, ms, ot, psum)
                while q:
                    emit_prep()
                if pending is not None:
                    epi_group(*pending)

            if repeat > 1:
                with tc.For_i(0, repeat, 1):
                    emit_body()
            else:
                emit_body()

            if timing_variant:
                dsb = out_pool.tile([P, 16], F32, name="dsb", tag="dsb")
                nc.any.memset(dsb, 1.0)
                nc.sync.dma_start(dummy_out, dsb)

    nc.compile()
    return nc


_NC_CACHE = {}


def _get_nc(M, K, N, **kw):
    key = (M, K, N, tuple(sorted(kw.items())))
    if key not in _NC_CACHE:
        _NC_CACHE[key] = build_nc(M, K, N, **kw)
    return _NC_CACHE[key]


LAST_RESULTS = None


def _bf16_trunc(a):
    """Sign-exact f32 -> bf16 truncation (keeps sign+exponent+7 mantissa)."""
    import ml_dtypes

    return (a.view(np.uint32) >> np.uint32(16)).astype(np.uint16).view(
        ml_dtypes.bfloat16
    )


def _u8_slice(a):
    """Sign-exact f32 -> top-byte u8 (sign + 7 exponent bits). -0.0 is
    normalized to +0.0 first so byte<128 <=> sign(v)=+1 matches v>=0."""
    a = a.copy()
    a[a == 0] = 0.0
    return (a.view(np.uint32) >> np.uint32(24)).astype(np.uint8)


def _pretile(shard_bf16):
    """[rows, K] bf16 -> pre-tiled [S*UG*P, KG*SUPER] so each (super s,
    k-group ug) DMA unit is one contiguous block."""
    rows, K = shard_bf16.shape
    S = rows // SUPER
    KT = K // P
    UG = KT // KG
    t = np.ascontiguousarray(shard_bf16.T)  # [K, rows]
    t = t.reshape(UG, KG, P, S, SUPER).transpose(3, 0, 2, 1, 4)
    return np.ascontiguousarray(t.reshape(S * UG * P, KG * SUPER))


def _bias_cols(bias_shard):
    """[N] -> [P, N/P] with bias_c[p, j] = bias[j*P + p]."""
    NT = bias_shard.shape[0] // P
    return np.ascontiguousarray(
        bias_shard.astype(np.float32).reshape(NT, P).T
    )


def make_in_maps(x, weight, bias):
    MS = x.shape[0] // M_SPLIT
    NS = weight.shape[0] // N_SPLIT
    xb = _u8_slice(np.ascontiguousarray(x, dtype=np.float32))
    wb = _u8_slice(np.ascontiguousarray(weight, dtype=np.float32))
    xTs = [_pretile(xb[mi * MS : (mi + 1) * MS]) for mi in range(M_SPLIT)]
    wTs = [_pretile(wb[ni * NS : (ni + 1) * NS]) for ni in range(N_SPLIT)]
    bcs = [
        _bias_cols(np.ascontiguousarray(bias[ni * NS : (ni + 1) * NS]))
        for ni in range(N_SPLIT)
    ]
    in_maps = []
    for c in range(N_CORES):
        mi, ni = divmod(c, N_SPLIT)
        in_maps.append(
            {"xT_shard": xTs[mi], "wT_shard": wTs[ni], "bias_c": bcs[ni]}
        )
    return in_maps


def kernel(x, weight, bias):
    global LAST_RESULTS
    x = np.asarray(x, dtype=np.float32)
    weight = np.asarray(weight, dtype=np.float32)
    bias = np.asarray(bias, dtype=np.float32)
    B, K = x.shape
    O = weight.shape[0]
    assert B % M_SPLIT == 0 and O % N_SPLIT == 0

    nc = _get_nc(B // M_SPLIT, K, O // N_SPLIT)
    in_maps = make_in_maps(x, weight, bias)

    last_exc = None
    for _attempt in range(3):
        try:
            res = run_bass_kernel_spmd(nc, in_maps, core_ids=list(range(N_CORES)))
            break
        except Exception as e:  # transient NRT/device wedges recover on retry
            last_exc = e
            os.environ.setdefault("NEURON_RT_RESET_CORES", "1")
    else:
        raise last_exc
    LAST_RESULTS = res

    MS = B // M_SPLIT
    NS = O // N_SPLIT
    out = np.empty((B, O), dtype=np.float32)
    for c in range(N_CORES):
        mi, ni = divmod(c, N_SPLIT)
        out[mi * MS : (mi + 1) * MS, ni * NS : (ni + 1) * NS] = res.results[c][
            "outT_shard"
        ].T
    return out


# revision 41
# speedup vs baseline: 1.0341x; 1.0341x over previous
"""Trainium2 Bass kernel for nn_BinaryLinear (binarized linear layer).

Computes: out = sign(x) @ sign(W).T + bias
  x: [8192, 4096] f32, W: [4096, 4096] f32, bias: [4096] f32 -> out [8192, 4096] f32
  sign(v) = +1 if v >= 0 else -1

Sharding: 4x2 grid over 8 NeuronCores - batch split 4 ways (2048 rows each),
W rows (out_features) split 2 ways (2048 each). Each core computes a disjoint
[2048, 2048] output block; no collectives.

Host-side staging (inside kernel(), part of sharding): each operand shard is
shipped K-major (transposed) as the f32 TOP BYTE (sign + 7 exponent bits;
-0.0 normalized to +0.0 first), pre-tiled so each DMA unit is one contiguous
256 KiB block ([128 partitions, 4 k-tiles, 512 rows]). byte < 128 <=> v >= 0,
so the device-side binarize sees exactly the signs the reference sees. This
cuts HBM input traffic 4x (32 MiB/core total vs 80) and removes the on-device
transpose entirely - the PE runs a pure DoubleRow fp8 matmul stream.

Device-side (per core), exact (rel err 0 vs the reference):
  1. Prep: DMA one 256 KiB u8 unit into a staging ring, DVE binarizes to
     fp8 +-0.5 in one op ((byte<128) - 0.5) straight into the resident
     K-major per-super operand tensors. No PE transposes, no PSUM round-trip.
     The DVE runs ONLY binarizes, so across loop passes the next pass's
     operand prep is never queued behind an end-of-pass epilogue op.
  2. Matmul: fp8 DoubleRow K-accumulated PE matmuls computing the TRANSPOSED
     output (out_features on partitions): psum = exact_int/4 (quarter-ints
     bounded by 1024 accumulate exactly in fp32 PSUM).
  3. Epilogue: single fused ACT op per tile: out = psum*4 + bias, with bias
     as a per-partition operand (out_features is the partition dim). DMA out;
     the host transposes each [N, M] shard back during unshard.
Blocks are ordered to retire w-super 0 and x-super 0 as early as possible so
the next loop pass's first binarizes get a long runway.
"""

import os

import numpy as np

import concourse.bacc as bacc
import concourse.mybir as mybir
import concourse.tile as tile
from concourse.alu_op_type import AluOpType
from concourse.bass_utils import run_bass_kernel_spmd

P = 128
N_CORES = 8
M_SPLIT = 4  # batch split
N_SPLIT = 2  # out_features split

# Full-problem shapes (hardcoded per harness contract)
BATCH = 8192
IN_FEATURES = 4096
OUT_FEATURES = 4096

F32 = mybir.dt.float32
BF16 = mybir.dt.bfloat16
FP8 = mybir.dt.float8e4

SUPER = 512  # rows per operand super == matmul moving free dim == PSUM bank
KG = 4  # k-tiles per prep unit (unit = contiguous [P, KG, SUPER] bf16)


def build_nc(
    M,
    K,
    N,
    n_cores=N_CORES,
    double_row=True,
    repeat=1,
    timing_variant=False,
    body_parts="all",  # "all" | "mm" | "prep" | "prep_nodma" | "all_nodma"
    stage_bufs=12,
    mm_bufs=8,
    out_bufs=6,
    kg=KG,  # k-tiles per prep unit (host _pretile must match module KG)
    dma_split=1,  # parallel dma_starts per stage unit (cuts per-unit latency)
    u8=True,  # operands shipped as 1-byte f32 top-byte slices (sign+exp)
    block_order="retire0",  # "retire0" | "natural"
    stage_eng="sync",  # engine queue issuing stage-load DMAs
    act_binarize=False,  # binarize w-supers 1.. on ACT (Sign -> +-1) to offload DVE
    swil=False,  # DoubleRowSwInterleave: host pre-interleaves the stationary
    # operand so LDWEIGHTS reads sequentially (stock DoubleRow reads the
    # weight columns reversed+interleaved, paying ~+72% on the load)
):
    """Build the per-core kernel. DRAM inputs (pre-tiled on host):
      xT_shard [M/SUPER * KT/KG * P, KG*SUPER] bf16   (moving operand)
      wT_shard [N/SUPER * KT/KG * P, KG*SUPER] bf16   (stationary operand)
      bias_c   [P, N/P] f32  (bias_c[p, j] = bias[j*P + p])
    -> outT_shard [N, M] f32  (transposed output block)
    """
    assert double_row, "v3 kernel is DoubleRow-only"
    assert K % (P * kg) == 0 and M % SUPER == 0 and N % SUPER == 0
    KT = K // P  # k-tiles (32)
    KP = KT // 2  # k-pairs per psum accumulation (16)
    UG = KT // kg  # prep units per super (8)
    MS_ = M // SUPER  # x supers / moving panels (4)
    NS_ = N // SUPER  # w supers (4)
    RB = SUPER // P  # o-tiles per w super (4)
    NT = N // P  # bias columns (16)

    IN_DT = mybir.dt.uint8 if u8 else BF16

    nc = bacc.Bacc(
        "TRN2", target_bir_lowering=False, debug=False, num_devices=n_cores
    )
    if timing_variant:
        xT_in = nc.dram_tensor("xT_int", [MS_ * UG * P, kg * SUPER], IN_DT).ap()
        wT_in = nc.dram_tensor("wT_int", [NS_ * UG * P, kg * SUPER], IN_DT).ap()
        b_in = nc.dram_tensor("b_int", [P, NT], F32).ap()
        out = nc.dram_tensor("outT_int", [N, M], F32).ap()
        dummy_out = nc.dram_tensor("dummy_out", [P, 16], F32, kind="ExternalOutput").ap()
    else:
        xT_in = nc.dram_tensor(
            "xT_shard", [MS_ * UG * P, kg * SUPER], IN_DT, kind="ExternalInput"
        ).ap()
        wT_in = nc.dram_tensor(
            "wT_shard", [NS_ * UG * P, kg * SUPER], IN_DT, kind="ExternalInput"
        ).ap()
        b_in = nc.dram_tensor("bias_c", [P, NT], F32, kind="ExternalInput").ap()
        out = nc.dram_tensor("outT_shard", [N, M], F32, kind="ExternalOutput").ap()

    with tile.TileContext(nc) as tc:
        with (
            tc.tile_pool(name="const", bufs=1) as const,
            tc.tile_pool(name="resid", bufs=1) as resid,
            tc.tile_pool(name="stage", bufs=stage_bufs) as stage_pool,
            tc.tile_pool(name="mm", bufs=mm_bufs, space="PSUM") as mm_pool,
            tc.tile_pool(name="outp", bufs=out_bufs) as out_pool,
        ):
            bias_sb = const.tile([P, NT], F32, name="bias_sb", tag="bias_sb")
            nc.sync.dma_start(bias_sb, b_in)

            xT = [
                resid.tile([P, KT, SUPER], FP8, name=f"xT{s}", tag=f"xT{s}")
                for s in range(MS_)
            ]
            if swil:
                # [P, k-pair, o-tile, interleaved (2*(127-o)+pair)] fp8
                wT = [
                    resid.tile(
                        [P, KP, RB, 2 * P], FP8, name=f"wT{s}", tag=f"wT{s}"
                    )
                    for s in range(NS_)
                ]
            else:
                wT = [
                    resid.tile([P, KT, SUPER], FP8, name=f"wT{s}", tag=f"wT{s}")
                    for s in range(NS_)
                ]

            if body_parts == "mm":
                for t in xT + wT:
                    nc.any.memset(t, 0.5)

            c128 = None
            if act_binarize:
                c128 = const.tile([P, 1], F32, name="c128", tag="c128")
                nc.any.memset(c128, 128.0)

            fixed_stage = None
            if body_parts in ("prep_nodma", "all_nodma"):
                fixed_stage = const.tile(
                    [P, kg * SUPER], IN_DT, name="fixed_stage", tag="fixed_stage"
                )
                nc.any.memset(fixed_stage, 1 if u8 else 0.25)

            def prep_unit(kind, s, ug):
                """Load unit (super s, k-group ug) and binarize to fp8 +-0.5
                into xT[s][:, ug*kg:(ug+1)*kg, :] (resp. wT)."""
                src_ap = xT_in if kind == "x" else wT_in
                dst = (xT if kind == "x" else wT)[s]
                r0 = (s * UG + ug) * P
                if fixed_stage is not None:
                    st = fixed_stage
                else:
                    st = stage_pool.tile(
                        [P, kg * SUPER], IN_DT, name="stage", tag="stage"
                    )
                    cw = kg * SUPER // dma_split
                    eng = getattr(nc, stage_eng)
                    for d in range(dma_split):
                        eng.dma_start(
                            st[:, d * cw : (d + 1) * cw],
                            src_ap[r0 : r0 + P, d * cw : (d + 1) * cw],
                        )
                if swil and kind == "w":
                    kp0 = ug * kg // 2
                    dst_sl = dst[:, kp0 : kp0 + kg // 2, :, :]
                else:
                    dst_sl = dst[:, ug * kg : (ug + 1) * kg, :]
                if u8:
                    if act_binarize and kind == "w" and s > 0:
                        # ACT path: Sign(128 - byte) = +-1 (w operands at +-1,
                        # x at +-0.5 -> psum = S/2, epilogue scale 2). Only
                        # w1.. go here: w0 must be ready at pass start, and
                        # ACT's queue tail is end-gated by the last epilogue.
                        nc.scalar.activation(
                            dst_sl,
                            st,
                            mybir.ActivationFunctionType.Sign,
                            scale=-1.0,
                            bias=c128[:, 0:1],
                        )
                    else:
                        # byte = f32 top byte; bit7 = sign: byte < 128 <=> v >= 0
                        nc.vector.tensor_scalar(
                            out=dst_sl,
                            in0=st,
                            scalar1=128,
                            scalar2=0.5,
                            op0=AluOpType.is_lt,
                            op1=AluOpType.subtract,
                        )
                else:
                    nc.vector.tensor_scalar(
                        out=dst_sl,
                        in0=st,
                        scalar1=0.0,
                        scalar2=0.5,
                        op0=AluOpType.is_ge,
                        op1=AluOpType.subtract,
                    )

            def mm_group(os_, ms, ot):
                """16 accumulating DR MMs for one [128(o), SUPER(b)] psum."""
                psum = mm_pool.tile([P, SUPER], F32, name="mmps", tag="mmps")
                for kp in range(KP):
                    if swil:
                        lhsT = wT[os_][:, kp, ot, :]
                        pm = mybir.MatmulPerfMode.DoubleRowSwInterleave
                    else:
                        lhsT = wT[os_][:, 2 * kp : 2 * kp + 2, ot * P : (ot + 1) * P]
                        pm = mybir.MatmulPerfMode.DoubleRow
                    nc.tensor.matmul(
                        psum,
                        lhsT=lhsT,
                        rhs=xT[ms][:, 2 * kp : 2 * kp + 2, :],
                        start=(kp == 0),
                        stop=(kp == KP - 1),
                        perf_mode=pm,
                    )
                return psum

            def epi_group(os_, ms, ot, psum):
                ob = out_pool.tile([P, SUPER], F32, name="ob", tag="ob")
                # psum holds exact_int/4 (or /2 for ACT-binarized +-1 w
                # supers); one fused op: out = psum*scale + bias (bias is
                # per-partition in this orientation)
                scl = 2.0 if (act_binarize and os_ > 0) else 4.0
                nc.scalar.activation(
                    ob,
                    psum,
                    mybir.ActivationFunctionType.Identity,
                    scale=scl,
                    bias=bias_sb[:, os_ * RB + ot : os_ * RB + ot + 1],
                )
                r0 = os_ * SUPER + ot * P
                nc.sync.dma_start(
                    out[r0 : r0 + P, ms * SUPER : (ms + 1) * SUPER], ob
                )

            # prep order: w0/x0 k-interleaved, then x1.. (needed by the first
            # block sweep), then w1..
            first_q = [
                (kind, 0, ug) for ug in range(UG) for kind in ("w", "x")
            ]
            rest_q = [("x", s, ug) for s in range(1, MS_) for ug in range(UG)] + [
                ("w", s, ug) for s in range(1, NS_) for ug in range(UG)
            ]
            prep_q_all = first_q + rest_q

            def emit_body():
                if body_parts in ("prep", "prep_nodma"):
                    for unit in prep_q_all:
                        prep_unit(*unit)
                    return
                if body_parts == "mm":
                    for os_ in range(NS_):
                        for ms in range(MS_):
                            for ot in range(RB):
                                psum = mm_group(os_, ms, ot)
                                epi_group(os_, ms, ot, psum)
                    return

                q = list(prep_q_all)
                totals = {}
                for kind, s, ug in q:
                    totals[(kind, s)] = totals.get((kind, s), 0) + 1
                done = {}

                def emit_prep():
                    kind, s, ug = q.pop(0)
                    prep_unit(kind, s, ug)
                    done[(kind, s)] = done.get((kind, s), 0) + 1

                def deps_met(keys):
                    return all(done.get(k, 0) == totals[k] for k in keys)

                # Retire w-super 0 and x-super 0 as early as possible: the
                # next loop pass's first MM blocks need them re-binarized,
                # and that binarize can only start once the last reader in
                # THIS pass is done.
                blocks = []
                if block_order == "retire0":
                    for ms in range(MS_):
                        for ot in range(RB):
                            blocks.append((0, ms, ot))
                    for os_ in range(1, NS_):
                        for ot in range(RB):
                            blocks.append((os_, 0, ot))
                    for os_ in range(1, NS_):
                        for ms in range(1, MS_):
                            for ot in range(RB):
                                blocks.append((os_, ms, ot))
                else:
                    for os_ in range(NS_):
                        for ms in range(MS_):
                            for ot in range(RB):
                                blocks.append((os_, ms, ot))

                per_block = (len(q) + len(blocks) - 1) // len(blocks)
                pending = None
                for os_, ms, ot in blocks:
                    need = [("w", os_), ("x", ms)]
                    while q and not deps_met(need):
                        emit_prep()
                    psum = mm_group(os_, ms, ot)
                    want = per_block
                    while q and want > 0:
                        emit_prep()
                        want -= 1
                    if pending is not None:
                        epi_group(*pending)
                    pending = (os_, ms, ot, psum)
                while q:
                    emit_prep()
                if pending is not None:
                    epi_group(*pending)

            if repeat > 1:
                with tc.For_i(0, repeat, 1):
                    emit_body()
            else:
                emit_body()

            if timing_variant:
                dsb = out_pool.tile([P, 16], F32, name="dsb", tag="dsb")
                nc.any.memset(dsb, 1.0)
                nc.sync.dma_start(dummy_out, dsb)

    nc.compile()
    return nc


_NC_CACHE = {}


def _get_nc(M, K, N, **kw):
    key = (M, K, N, tuple(sorted(kw.items())))
    if key not in _NC_CACHE:
        _NC_CACHE[key] = build_nc(M, K, N, **kw)
    return _NC_CACHE[key]


LAST_RESULTS = None


def _bf16_trunc(a):
    """Sign-exact f32 -> bf16 truncation (keeps sign+exponent+7 mantissa)."""
    import ml_dtypes

    return (a.view(np.uint32) >> np.uint32(16)).astype(np.uint16).view(
        ml_dtypes.bfloat16
    )


def _u8_slice(a):
    """Sign-exact f32 -> top-byte u8 (sign + 7 exponent bits). -0.0 is
    normalized to +0.0 first so byte<128 <=> sign(v)=+1 matches v>=0."""
    a = a.copy()
    a[a == 0] = 0.0
    return (a.view(np.uint32) >> np.uint32(24)).astype(np.uint8)


def _pretile(shard_bf16):
    """[rows, K] bf16 -> pre-tiled [S*UG*P, KG*SUPER] so each (super s,
    k-group ug) DMA unit is one contiguous block."""
    rows, K = shard_bf16.shape
    S = rows // SUPER
    KT = K // P
    UG = KT // KG
    t = np.ascontiguousarray(shard_bf16.T)  # [K, rows]
    t = t.reshape(UG, KG, P, S, SUPER).transpose(3, 0, 2, 1, 4)
    return np.ascontiguousarray(t.reshape(S * UG * P, KG * SUPER))


def _pretile_w_swil(shard_u8):
    """[rows, K] u8 -> pre-tiled stationary units whose free order is
    [k-pair-local, o-tile, 2*(127-o)+pair] (DoubleRowSwInterleave layout)."""
    rows, K = shard_u8.shape
    S = rows // SUPER
    RB = SUPER // P
    KPt = K // (2 * P)
    UG = (K // P) // KG
    t = shard_u8.reshape(S, RB, P, KPt, 2, P)  # [s, ot, o, kp, pair, p]
    t = t[:, :, ::-1, :, :, :]  # o reversed
    t = t.transpose(0, 3, 5, 1, 2, 4)  # [s, kp, p, ot, o_rev, pair]
    t = np.ascontiguousarray(t).reshape(S, UG, KG // 2, P, RB, 2 * P)
    t = t.transpose(0, 1, 3, 2, 4, 5)  # [s, ug, p, kp_local, ot, io]
    return np.ascontiguousarray(t.reshape(S * UG * P, KG * SUPER))


def _bias_cols(bias_shard):
    """[N] -> [P, N/P] with bias_c[p, j] = bias[j*P + p]."""
    NT = bias_shard.shape[0] // P
    return np.ascontiguousarray(
        bias_shard.astype(np.float32).reshape(NT, P).T
    )


def make_in_maps(x, weight, bias):
    MS = x.shape[0] // M_SPLIT
    NS = weight.shape[0] // N_SPLIT
    xb = _u8_slice(np.ascontiguousarray(x, dtype=np.float32))
    wb = _u8_slice(np.ascontiguousarray(weight, dtype=np.float32))
    xTs = [_pretile(xb[mi * MS : (mi + 1) * MS]) for mi in range(M_SPLIT)]
    wTs = [_pretile(wb[ni * NS : (ni + 1) * NS]) for ni in range(N_SPLIT)]
    bcs = [
        _bias_cols(np.ascontiguousarray(bias[ni * NS : (ni + 1) * NS]))
        for ni in range(N_SPLIT)
    ]
    in_maps = []
    for c in range(N_CORES):
        mi, ni = divmod(c, N_SPLIT)
        in_maps.append(
            {"xT_shard": xTs[mi], "wT_shard": wTs[ni], "bias_c": bcs[ni]}
        )
    return in_maps


def kernel(x, weight, bias):
    global LAST_RESULTS
    x = np.asarray(x, dtype=np.float32)
    weight = np.asarray(weight, dtype=np.float32)
    bias = np.asarray(bias, dtype=np.float32)
    B, K = x.shape
    O = weight.shape[0]
    assert B % M_SPLIT == 0 and O % N_SPLIT == 0

    nc = _get_nc(B // M_SPLIT, K, O // N_SPLIT)
    in_maps = make_in_maps(x, weight, bias)

    last_exc = None
    for _attempt in range(3):
        try:
            res = run_bass_kernel_spmd(nc, in_maps, core_ids=list(range(N_CORES)))
            break
        except Exception as e:  # transient NRT/device wedges recover on retry
            last_exc = e
            os.environ.setdefault("NEURON_RT_RESET_CORES", "1")
    else:
        raise last_exc
    LAST_RESULTS = res

    MS = B // M_SPLIT
    NS = O // N_SPLIT
    out = np.empty((B, O), dtype=np.float32)
    for c in range(N_CORES):
        mi, ni = divmod(c, N_SPLIT)
        out[mi * MS : (mi + 1) * MS, ni * NS : (ni + 1) * NS] = res.results[c][
            "outT_shard"
        ].T
    return out
